# revision 10
# baseline (speedup 1.0000x reference)
"""DeepSeek-V2 MoE layer (T=2048, H=2048, I=1408, E=8, top-2) on 8 TRN2 cores.

Strategy: expert parallelism. The router (67 MFLOP, 0.06% of total work) runs
on the host to produce the token->expert dispatch; each NeuronCore runs one
expert's gate/up/down GEMMs over the tokens routed to it (padded to a fixed
capacity C), with the top-2 combine weight folded into the output. The host
scatter-adds the per-expert outputs back into the full [T, H] output.

Matmuls run as fp32r (TF32-like, 11 mantissa bits, full PE rate). Inputs are
pre-rounded to the fp32r-representable subset on the host so tiles can be
DMA'd directly (the BIR verifier requires fp32r matmul operands to come from
a rounding producer or be declared fp32r end-to-end).
"""
import sys

_TRN = "/opt/trn_rl_repo"
if _TRN not in sys.path:
    sys.path.insert(0, _TRN)

import numpy as np

import concourse.bacc as bacc
import concourse.mybir as mybir
import concourse.tile as tile
from concourse import bass_utils

T, H, I, E = 2048, 2048, 1408, 8
C = 576                       # per-expert token capacity (actual max count: 545)
NT, NI = H // 128, I // 128   # 16, 11
NCT = (C + 127) // 128        # 5 c-blocks; the last one is partial
C_LAST = C - 128 * (NCT - 1)  # 64
F32 = mybir.dt.float32
SPLITS = ((0, 320), (320, 256))   # phase-A free-dim split: single-bank, >=256

# Matmul operand dtype: "f32r" (TF32-like, ~2.5e-4 rel err) or "f16"
# (halves weight DMA, ~1e-3 rel err).
QUANT = "f32r"

_CACHE = {}


def _round_f32r(x: np.ndarray) -> np.ndarray:
    """Round fp32 to the fp32r-representable subset (RNE to 11 mantissa bits)."""
    u = np.ascontiguousarray(x, dtype=np.float32).view(np.uint32).astype(np.uint64)
    u = u + 0x7FF + ((u >> 12) & 1)
    return (u & np.uint64(0xFFFFF000)).astype(np.uint32).view(np.float32)


if QUANT == "f32r":
    MMDT = mybir.dt.float32r
    _quant = _round_f32r
else:
    MMDT = mybir.dt.float16
    def _quant(x):
        return np.ascontiguousarray(x, dtype=np.float32).astype(np.float16)


def _build():
    nc = bacc.Bacc("TRN2", target_bir_lowering=False, debug=False, num_devices=8)
    xt_d = nc.dram_tensor("xt", [H, C], MMDT, kind="ExternalInput").ap()
    wg_d = nc.dram_tensor("wg", [NI, 128, H], MMDT, kind="ExternalInput").ap()
    wu_d = nc.dram_tensor("wu", [NI, 128, H], MMDT, kind="ExternalInput").ap()
    wd_d = nc.dram_tensor("wd", [I, H], MMDT, kind="ExternalInput").ap()
    cmb_d = nc.dram_tensor("cmb", [C, 1], F32, kind="ExternalInput").ap()
    y_d = nc.dram_tensor("y", [C, H], F32, kind="ExternalOutput").ap()

    with tile.TileContext(nc) as tc:
        with (
            tc.tile_pool(name="xtp", bufs=1) as xtp,
            tc.tile_pool(name="wp", bufs=2) as wp,
            tc.tile_pool(name="htp", bufs=NI) as htp,
            tc.tile_pool(name="wdp", bufs=NI) as wdp,
            tc.tile_pool(name="mp", bufs=2) as mp,
            tc.tile_pool(name="op", bufs=1) as op,
        ):
            # Front loads, interleaved so the PE can start as early as
            # possible: the first phase-A iteration's weights arrive in
            # 512-column chunks alongside the first x^T tiles.
            xt = xtp.tile([128, NT, C], MMDT, tag="xt")
            wgt0 = wp.tile([128, H], MMDT, tag="wg", name="wgt0")
            wut0 = wp.tile([128, H], MMDT, tag="wu", name="wut0")
            order = [("g", 0), ("x", 0), ("g", 1), ("x", 1), ("g", 2), ("x", 2),
                     ("g", 3), ("x", 3), ("u", 0), ("x", 4), ("u", 1), ("x", 5),
                     ("u", 2), ("x", 6), ("u", 3), ("x", 7)] + \
                    [("x", t) for t in range(8, NT)]
            for kind, k in order:
                if kind == "g":
                    nc.sync.dma_start(wgt0[:, k * 512:(k + 1) * 512],
                                      wg_d[0, :, k * 512:(k + 1) * 512])
                elif kind == "u":
                    nc.sync.dma_start(wut0[:, k * 512:(k + 1) * 512],
                                      wu_d[0, :, k * 512:(k + 1) * 512])
                else:
                    nc.sync.dma_start(xt[:, k, :], xt_d[k * 128:(k + 1) * 128, :])
            cmb = xtp.tile([128, NCT], F32, tag="cmb")
            for c in range(NCT):
                w = 128 if c < NCT - 1 else C_LAST
                nc.sync.dma_start(cmb[:w, c:c + 1], cmb_d[c * 128:c * 128 + w, :])

            wd_tiles = []
            ht_tiles = []

            # Phase A: hT[i] = silu(Wg[:,i]^T x^T) * (Wu[:,i]^T x^T), [128, C]
            # Each matmul output must stay inside one 2KB PSUM bank and
            # start=True clears the whole bank, so the C free dim is split
            # into two single-bank tiles (both >=256 keeps fp32r full rate).
            with tc.tile_pool(name="psA", bufs=2, space="PSUM") as psA:
                for i in range(NI):
                    if i == 0:
                        wg_sl = lambda t: wgt0[:, t * 128:(t + 1) * 128]
                        wu_sl = lambda t: wut0[:, t * 128:(t + 1) * 128]
                    else:
                        wgt = wp.tile([128, H], MMDT, tag="wg")
                        wut = wp.tile([128, H], MMDT, tag="wu")
                        nc.sync.dma_start(wgt[:], wg_d[i])
                        nc.sync.dma_start(wut[:], wu_d[i])
                        wg_sl = lambda t, w=wgt: w[:, t * 128:(t + 1) * 128]
                        wu_sl = lambda t, w=wut: w[:, t * 128:(t + 1) * 128]
                    pg = [psA.tile([128, w], F32, tag=f"pg{k}", name=f"pg{k}_{i}")
                          for k, (_, w) in enumerate(SPLITS)]
                    pu = [psA.tile([128, w], F32, tag=f"pu{k}", name=f"pu{k}_{i}")
                          for k, (_, w) in enumerate(SPLITS)]
                    for t in range(NT):
                        for k, (lo, w) in enumerate(SPLITS):
                            nc.tensor.matmul(pg[k][:], wg_sl(t),
                                             xt[:, t, lo:lo + w],
                                             start=(t == 0), stop=(t == NT - 1))
                    for t in range(NT):
                        for k, (lo, w) in enumerate(SPLITS):
                            nc.tensor.matmul(pu[k][:], wu_sl(t),
                                             xt[:, t, lo:lo + w],
                                             start=(t == 0), stop=(t == NT - 1))
                    tmp = mp.tile([128, C], F32, tag="tmp")
                    ht = htp.tile([128, C], MMDT, tag="ht")
                    for k, (lo, w) in enumerate(SPLITS):
                        nc.scalar.activation(tmp[:, lo:lo + w], pg[k][:],
                                             mybir.ActivationFunctionType.Silu)
                        nc.vector.tensor_mul(ht[:, lo:lo + w], tmp[:, lo:lo + w],
                                             pu[k][:])
                    ht_tiles.append(ht)

                    # Trickle-in the down-proj weights during phase A.
                    wdt = wdp.tile([128, H], MMDT, tag="wd")
                    nc.sync.dma_start(wdt[:], wd_d[i * 128:(i + 1) * 128, :])
                    wd_tiles.append(wdt)

            # Phase B: y[c-block] = sum_i hT[i][:, c-block]^T @ Wd[i], scaled
            # by the per-token combine weight.
            with tc.tile_pool(name="psB", bufs=2, space="PSUM") as psB:
                for c in range(NCT):
                    cw = 128 if c < NCT - 1 else C_LAST
                    po = psB.tile([cw, H], F32, tag="po", name=f"po_{c}")
                    cs = slice(c * 128, c * 128 + cw)
                    for i in range(NI):
                        for n in range(4):
                            ns = slice(n * 512, (n + 1) * 512)
                            nc.tensor.matmul(po[:, ns], ht_tiles[i][:, cs],
                                             wd_tiles[i][:, ns],
                                             start=(i == 0), stop=(i == NI - 1))
                    ot = op.tile([cw, H], F32, tag="ot", name=f"ot_{c}")
                    for n in range(4):
                        ns = slice(n * 512, (n + 1) * 512)
                        nc.vector.tensor_scalar_mul(ot[:, ns], po[:, ns],
                                                    cmb[:cw, c:c + 1])
                        nc.sync.dma_start(y_d[cs, ns], ot[:, ns])

    nc.compile()
    return nc


def _route(X: np.ndarray, Wr: np.ndarray):
    """Host router: top-2 of softmax(X @ Wr), renormalized over the top-2."""
    logits = X.astype(np.float64) @ Wr.astype(np.float64)
    order = np.argsort(-logits, axis=1)
    top1, top2 = order[:, 0], order[:, 1]
    rows = np.arange(len(X))
    l1, l2 = logits[rows, top1], logits[rows, top2]
    e21 = np.exp(l2 - l1)
    w1 = 1.0 / (1.0 + e21)
    w2 = e21 / (1.0 + e21)
    return top1, top2, w1.astype(np.float32), w2.astype(np.float32)


def _reference_numpy(hidden_states, w_router, w_gate, w_up, w_down):
    X = np.asarray(hidden_states, np.float32)
    top1, top2, w1, w2 = _route(X, np.asarray(w_router, np.float32))
    out = np.zeros((T, H), np.float32)
    for e in range(E):
        sel = np.where((top1 == e) | (top2 == e))[0]
        if len(sel) == 0:
            continue
        w = np.where(top1[sel] == e, w1[sel], w2[sel])[:, None]
        x = X[sel]
        h = (x @ w_gate[e])
        h = (h / (1.0 + np.exp(-h))) * (x @ w_up[e]) * w
        out[sel] += h @ w_down[e]
    return out


def _make_in_maps(X, Wg, Wu, Wd, sels, wts):
    Xq = _quant(X)
    in_maps = []
    for e in range(E):
        sel, w = sels[e], wts[e]
        n = len(sel)
        xt = np.zeros((H, C), Xq.dtype)
        xt[:, :n] = Xq[sel].T
        cmb = np.zeros((C, 1), np.float32)
        cmb[:n, 0] = w
        wg_sw = (_quant(Wg[e]).reshape(NT, 128, NI, 128)
                 .transpose(2, 1, 0, 3).reshape(NI, 128, H))
        wu_sw = (_quant(Wu[e]).reshape(NT, 128, NI, 128)
                 .transpose(2, 1, 0, 3).reshape(NI, 128, H))
        wd_q = _quant(Wd[e])
        in_maps.append({
            "xt": np.ascontiguousarray(xt),
            "wg": np.ascontiguousarray(wg_sw),
            "wu": np.ascontiguousarray(wu_sw),
            "wd": wd_q,
            "cmb": cmb,
        })
    return in_maps


def kernel(hidden_states, w_router, w_gate, w_up, w_down):
    X = np.ascontiguousarray(hidden_states, dtype=np.float32)
    Wr = np.ascontiguousarray(w_router, dtype=np.float32)
    Wg = np.ascontiguousarray(w_gate, dtype=np.float32)
    Wu = np.ascontiguousarray(w_up, dtype=np.float32)
    Wd = np.ascontiguousarray(w_down, dtype=np.float32)

    top1, top2, w1, w2 = _route(X, Wr)
    sels, wts = [], []
    for e in range(E):
        sel = np.where((top1 == e) | (top2 == e))[0]
        sels.append(sel)
        wts.append(np.where(top1[sel] == e, w1[sel], w2[sel]))
    if max(len(s) for s in sels) > C:
        # Capacity overflow (cannot happen for the reference input
        # distribution); fall back to a host implementation.
        return _reference_numpy(X, Wr, Wg, Wu, Wd)

    if "nc" not in _CACHE:
        _CACHE["nc"] = _build()
    nc = _CACHE["nc"]

    in_maps = _make_in_maps(X, Wg, Wu, Wd, sels, wts)
    res = bass_utils.run_bass_kernel_spmd(nc, in_maps, list(range(E)))

    out = np.zeros((T, H), np.float32)
    for e in range(E):
        sel = sels[e]
        out[sel] += res.results[e]["y"][:len(sel)]
    return out


# revision 11
# speedup vs baseline: 1.1394x; 1.1394x over previous
"""DeepSeek-V2 MoE layer (T=2048, H=2048, I=1408, E=8, top-2) on 8 TRN2 cores.

Strategy: expert parallelism. The router (67 MFLOP, 0.06% of total work) runs
on the host to produce the token->expert dispatch; each NeuronCore runs one
expert's gate/up/down GEMMs over the tokens routed to it (padded to a fixed
capacity C), with the top-2 combine weight folded into the output. The host
scatter-adds the per-expert outputs back into the full [T, H] output.

Matmuls run as fp32r (TF32-like, 11 mantissa bits, full PE rate). Inputs are
pre-rounded to the fp32r-representable subset on the host so tiles can be
DMA'd directly (the BIR verifier requires fp32r matmul operands to come from
a rounding producer or be declared fp32r end-to-end).
"""
import sys

_TRN = "/opt/trn_rl_repo"
if _TRN not in sys.path:
    sys.path.insert(0, _TRN)

import numpy as np

import concourse.bacc as bacc
import concourse.mybir as mybir
import concourse.tile as tile
from concourse import bass_utils

T, H, I, E = 2048, 2048, 1408, 8
C = 576                       # per-expert token capacity (actual max count: 545)
NT, NI = H // 128, I // 128   # 16, 11
NCT = (C + 127) // 128        # 5 c-blocks; the last one is partial
C_LAST = C - 128 * (NCT - 1)  # 64
F32 = mybir.dt.float32
SPLITS = ((0, 320), (320, 256))   # phase-A free-dim split: single-bank, >=256

# Gate/up path (xt, wg, wu) runs in fp16: phase A is otherwise HBM-bound
# (fp32-class weights need ~390 GB/s vs ~358 available). The down
# projection (ht, wd) stays fp32r (TF32-like) to keep the final GEMM
# accurate.
XDT = mybir.dt.float16
DDT = mybir.dt.float32r

_CACHE = {}


def _round_f32r(x: np.ndarray) -> np.ndarray:
    """Round fp32 to the fp32r-representable subset (RNE to 11 mantissa bits)."""
    u = np.ascontiguousarray(x, dtype=np.float32).view(np.uint32).astype(np.uint64)
    u = u + 0x7FF + ((u >> 12) & 1)
    return (u & np.uint64(0xFFFFF000)).astype(np.uint32).view(np.float32)


def _quant_x(x):
    return np.ascontiguousarray(x, dtype=np.float32).astype(np.float16)


_quant_wd = _round_f32r


def _build():
    nc = bacc.Bacc("TRN2", target_bir_lowering=False, debug=False, num_devices=8)
    xt_d = nc.dram_tensor("xt", [H, C], XDT, kind="ExternalInput").ap()
    wg_d = nc.dram_tensor("wg", [NI, 128, H], XDT, kind="ExternalInput").ap()
    wu_d = nc.dram_tensor("wu", [NI, 128, H], XDT, kind="ExternalInput").ap()
    wd_d = nc.dram_tensor("wd", [I, H], DDT, kind="ExternalInput").ap()
    cmb_d = nc.dram_tensor("cmb", [C, 1], F32, kind="ExternalInput").ap()
    y_d = nc.dram_tensor("y", [C, H], F32, kind="ExternalOutput").ap()

    with tile.TileContext(nc) as tc:
        with (
            tc.tile_pool(name="xtp", bufs=1) as xtp,
            tc.tile_pool(name="wp", bufs=3) as wp,
            tc.tile_pool(name="htp", bufs=NI) as htp,
            tc.tile_pool(name="wdp", bufs=NI) as wdp,
            tc.tile_pool(name="mp", bufs=2) as mp,
            tc.tile_pool(name="op", bufs=2) as op,
        ):
            # Front loads, interleaved so the PE can start as early as
            # possible: the first phase-A iteration's weights arrive in
            # 512-column chunks alongside the first x^T tiles.
            xt = xtp.tile([128, NT, C], XDT, tag="xt")
            wgt0 = wp.tile([128, H], XDT, tag="wg", name="wgt0")
            wut0 = wp.tile([128, H], XDT, tag="wu", name="wut0")
            order = [("g", 0), ("x", 0), ("g", 1), ("x", 1), ("g", 2), ("x", 2),
                     ("g", 3), ("x", 3), ("u", 0), ("x", 4), ("u", 1), ("x", 5),
                     ("u", 2), ("x", 6), ("u", 3), ("x", 7)] + \
                    [("x", t) for t in range(8, NT)]
            for kind, k in order:
                if kind == "g":
                    nc.sync.dma_start(wgt0[:, k * 512:(k + 1) * 512],
                                      wg_d[0, :, k * 512:(k + 1) * 512])
                elif kind == "u":
                    nc.sync.dma_start(wut0[:, k * 512:(k + 1) * 512],
                                      wu_d[0, :, k * 512:(k + 1) * 512])
                else:
                    nc.sync.dma_start(xt[:, k, :], xt_d[k * 128:(k + 1) * 128, :])
            cmb = xtp.tile([128, NCT], F32, tag="cmb")
            for c in range(NCT):
                w = 128 if c < NCT - 1 else C_LAST
                nc.sync.dma_start(cmb[:w, c:c + 1], cmb_d[c * 128:c * 128 + w, :])

            wd_tiles = []
            ht_tiles = []

            # Phase A: hT[i] = silu(Wg[:,i]^T x^T) * (Wu[:,i]^T x^T), [128, C]
            # Each matmul output must stay inside one 2KB PSUM bank and
            # start=True clears the whole bank, so the C free dim is split
            # into two single-bank tiles (both >=256 keeps fp32r full rate).
            with tc.tile_pool(name="psA", bufs=2, space="PSUM") as psA:
                for i in range(NI):
                    if i == 0:
                        wg_sl = lambda t: wgt0[:, t * 128:(t + 1) * 128]
                        wu_sl = lambda t: wut0[:, t * 128:(t + 1) * 128]
                    else:
                        wgt = wp.tile([128, H], XDT, tag="wg")
                        wut = wp.tile([128, H], XDT, tag="wu")
                        nc.sync.dma_start(wgt[:], wg_d[i])
                        nc.sync.dma_start(wut[:], wu_d[i])
                        wg_sl = lambda t, w=wgt: w[:, t * 128:(t + 1) * 128]
                        wu_sl = lambda t, w=wut: w[:, t * 128:(t + 1) * 128]
                    pg = [psA.tile([128, w], F32, tag=f"pg{k}", name=f"pg{k}_{i}")
                          for k, (_, w) in enumerate(SPLITS)]
                    pu = [psA.tile([128, w], F32, tag=f"pu{k}", name=f"pu{k}_{i}")
                          for k, (_, w) in enumerate(SPLITS)]
                    for t in range(NT):
                        for k, (lo, w) in enumerate(SPLITS):
                            nc.tensor.matmul(pg[k][:], wg_sl(t),
                                             xt[:, t, lo:lo + w],
                                             start=(t == 0), stop=(t == NT - 1))
                    for t in range(NT):
                        for k, (lo, w) in enumerate(SPLITS):
                            nc.tensor.matmul(pu[k][:], wu_sl(t),
                                             xt[:, t, lo:lo + w],
                                             start=(t == 0), stop=(t == NT - 1))
                    tmp = mp.tile([128, C], F32, tag="tmp")
                    ht = htp.tile([128, C], DDT, tag="ht")
                    for k, (lo, w) in enumerate(SPLITS):
                        nc.scalar.activation(tmp[:, lo:lo + w], pg[k][:],
                                             mybir.ActivationFunctionType.Silu)
                        nc.vector.tensor_mul(ht[:, lo:lo + w], tmp[:, lo:lo + w],
                                             pu[k][:])
                    ht_tiles.append(ht)

                    # Trickle-in the down-proj weights during phase A.
                    wdt = wdp.tile([128, H], DDT, tag="wd")
                    nc.sync.dma_start(wdt[:], wd_d[i * 128:(i + 1) * 128, :])
                    wd_tiles.append(wdt)

            # Phase B: y[c-block] = sum_i hT[i][:, c-block]^T @ Wd[i], scaled
            # by the per-token combine weight.
            with tc.tile_pool(name="psB", bufs=2, space="PSUM") as psB:
                for c in range(NCT):
                    cw = 128 if c < NCT - 1 else C_LAST
                    po = psB.tile([cw, H], F32, tag="po", name=f"po_{c}")
                    cs = slice(c * 128, c * 128 + cw)
                    for i in range(NI):
                        for n in range(4):
                            ns = slice(n * 512, (n + 1) * 512)
                            nc.tensor.matmul(po[:, ns], ht_tiles[i][:, cs],
                                             wd_tiles[i][:, ns],
                                             start=(i == 0), stop=(i == NI - 1))
                    ot = op.tile([cw, H], F32, tag="ot", name=f"ot_{c}")
                    for n in range(4):
                        ns = slice(n * 512, (n + 1) * 512)
                        nc.vector.tensor_scalar_mul(ot[:, ns], po[:, ns],
                                                    cmb[:cw, c:c + 1])
                        nc.sync.dma_start(y_d[cs, ns], ot[:, ns])

    nc.compile()
    return nc


def _route(X: np.ndarray, Wr: np.ndarray):
    """Host router: top-2 of softmax(X @ Wr), renormalized over the top-2."""
    logits = X.astype(np.float64) @ Wr.astype(np.float64)
    order = np.argsort(-logits, axis=1)
    top1, top2 = order[:, 0], order[:, 1]
    rows = np.arange(len(X))
    l1, l2 = logits[rows, top1], logits[rows, top2]
    e21 = np.exp(l2 - l1)
    w1 = 1.0 / (1.0 + e21)
    w2 = e21 / (1.0 + e21)
    return top1, top2, w1.astype(np.float32), w2.astype(np.float32)


def _reference_numpy(hidden_states, w_router, w_gate, w_up, w_down):
    X = np.asarray(hidden_states, np.float32)
    top1, top2, w1, w2 = _route(X, np.asarray(w_router, np.float32))
    out = np.zeros((T, H), np.float32)
    for e in range(E):
        sel = np.where((top1 == e) | (top2 == e))[0]
        if len(sel) == 0:
            continue
        w = np.where(top1[sel] == e, w1[sel], w2[sel])[:, None]
        x = X[sel]
        h = (x @ w_gate[e])
        h = (h / (1.0 + np.exp(-h))) * (x @ w_up[e]) * w
        out[sel] += h @ w_down[e]
    return out


def _make_in_maps(X, Wg, Wu, Wd, sels, wts):
    Xq = _quant_x(X)
    in_maps = []
    for e in range(E):
        sel, w = sels[e], wts[e]
        n = len(sel)
        xt = np.zeros((H, C), Xq.dtype)
        xt[:, :n] = Xq[sel].T
        cmb = np.zeros((C, 1), np.float32)
        cmb[:n, 0] = w
        wg_sw = (_quant_x(Wg[e]).reshape(NT, 128, NI, 128)
                 .transpose(2, 1, 0, 3).reshape(NI, 128, H))
        wu_sw = (_quant_x(Wu[e]).reshape(NT, 128, NI, 128)
                 .transpose(2, 1, 0, 3).reshape(NI, 128, H))
        wd_q = _quant_wd(Wd[e])
        in_maps.append({
            "xt": np.ascontiguousarray(xt),
            "wg": np.ascontiguousarray(wg_sw),
            "wu": np.ascontiguousarray(wu_sw),
            "wd": wd_q,
            "cmb": cmb,
        })
    return in_maps


def kernel(hidden_states, w_router, w_gate, w_up, w_down):
    X = np.ascontiguousarray(hidden_states, dtype=np.float32)
    Wr = np.ascontiguousarray(w_router, dtype=np.float32)
    Wg = np.ascontiguousarray(w_gate, dtype=np.float32)
    Wu = np.ascontiguousarray(w_up, dtype=np.float32)
    Wd = np.ascontiguousarray(w_down, dtype=np.float32)

    top1, top2, w1, w2 = _route(X, Wr)
    sels, wts = [], []
    for e in range(E):
        sel = np.where((top1 == e) | (top2 == e))[0]
        sels.append(sel)
        wts.append(np.where(top1[sel] == e, w1[sel], w2[sel]))
    if max(len(s) for s in sels) > C:
        # Capacity overflow (cannot happen for the reference input
        # distribution); fall back to a host implementation.
        return _reference_numpy(X, Wr, Wg, Wu, Wd)

    if "nc" not in _CACHE:
        _CACHE["nc"] = _build()
    nc = _CACHE["nc"]

    in_maps = _make_in_maps(X, Wg, Wu, Wd, sels, wts)
    res = bass_utils.run_bass_kernel_spmd(nc, in_maps, list(range(E)))

    out = np.zeros((T, H), np.float32)
    for e in range(E):
        sel = sels[e]
        out[sel] += res.results[e]["y"][:len(sel)]
    return out


# revision 12
# speedup vs baseline: 1.2214x; 1.0719x over previous
"""DeepSeek-V2 MoE layer (T=2048, H=2048, I=1408, E=8, top-2) on 8 TRN2 cores.

Strategy: expert parallelism. The router (67 MFLOP, 0.06% of total work) runs
on the host to produce the token->expert dispatch; each NeuronCore runs one
expert's gate/up/down GEMMs over the tokens routed to it (padded to a fixed
capacity C), with the top-2 combine weight folded into the output. The host
scatter-adds the per-expert outputs back into the full [T, H] output.

Matmuls run as fp32r (TF32-like, 11 mantissa bits, full PE rate). Inputs are
pre-rounded to the fp32r-representable subset on the host so tiles can be
DMA'd directly (the BIR verifier requires fp32r matmul operands to come from
a rounding producer or be declared fp32r end-to-end).
"""
import sys

_TRN = "/opt/trn_rl_repo"
if _TRN not in sys.path:
    sys.path.insert(0, _TRN)

import numpy as np

import concourse.bacc as bacc
import concourse.mybir as mybir
import concourse.tile as tile
from concourse import bass_utils

T, H, I, E = 2048, 2048, 1408, 8
C = 576                       # per-expert token capacity (actual max count: 545)
NT, NI = H // 128, I // 128   # 16, 11
NCT = (C + 127) // 128        # 5 c-blocks; the last one is partial
C_LAST = C - 128 * (NCT - 1)  # 64
F32 = mybir.dt.float32
SPLITS = ((0, 320), (320, 256))   # phase-A free-dim split: single-bank, >=256

# All matmul operands run in fp16 (10 mantissa bits, full PE rate, FWL
# background weight loads): fp32-class weights would make phase A
# HBM-bound (~390 GB/s needed vs ~358 available) and fp32r stationaries
# serialize their weight loads. fp32 accumulation throughout; measured
# end-to-end rel err ~1e-3 absmax-relative.
XDT = mybir.dt.float16
DDT = mybir.dt.float16

_CACHE = {}


def _round_f32r(x: np.ndarray) -> np.ndarray:
    """Round fp32 to the fp32r-representable subset (RNE to 11 mantissa bits)."""
    u = np.ascontiguousarray(x, dtype=np.float32).view(np.uint32).astype(np.uint64)
    u = u + 0x7FF + ((u >> 12) & 1)
    return (u & np.uint64(0xFFFFF000)).astype(np.uint32).view(np.float32)


def _quant_x(x):
    return np.ascontiguousarray(x, dtype=np.float32).astype(np.float16)


def _quant_wd(x):
    return np.ascontiguousarray(x, dtype=np.float32).astype(np.float16)


def _build():
    nc = bacc.Bacc("TRN2", target_bir_lowering=False, debug=False, num_devices=8)
    xt_d = nc.dram_tensor("xt", [128, NT * C], XDT, kind="ExternalInput").ap()
    wg_d = nc.dram_tensor("wg", [NI, 128, H], XDT, kind="ExternalInput").ap()
    wu_d = nc.dram_tensor("wu", [NI, 128, H], XDT, kind="ExternalInput").ap()
    wd_d = nc.dram_tensor("wd", [I, H], DDT, kind="ExternalInput").ap()
    cmb_d = nc.dram_tensor("cmb", [C, 1], F32, kind="ExternalInput").ap()
    y_d = nc.dram_tensor("y", [C, H], F32, kind="ExternalOutput").ap()

    with tile.TileContext(nc) as tc:
        with (
            tc.tile_pool(name="xtp", bufs=1) as xtp,
            tc.tile_pool(name="wp", bufs=3) as wp,
            tc.tile_pool(name="htp", bufs=NI) as htp,
            tc.tile_pool(name="wdp", bufs=NI) as wdp,
            tc.tile_pool(name="mp", bufs=2) as mp,
            tc.tile_pool(name="op", bufs=2) as op,
        ):
            # Front loads, interleaved so the PE can start as early as
            # possible: the first phase-A iteration's weights arrive in
            # 512-column chunks alongside the first x^T tiles.
            xt = xtp.tile([128, NT, C], XDT, tag="xt")
            xt_flat = xt.rearrange("p t c -> p (t c)")
            wgt0 = wp.tile([128, H], XDT, tag="wg", name="wgt0")
            wut0 = wp.tile([128, H], XDT, tag="wu", name="wut0")
            # x^T arrives in 8 two-h-block chunks; the first iteration's
            # weights arrive in 1024-column chunks, interleaved.
            order = [("g", 0), ("x", 0), ("g", 1), ("x", 1), ("u", 0), ("x", 2),
                     ("u", 1), ("x", 3)] + [("x", b) for b in range(4, 8)]
            XB = 2 * C  # elements per xt chunk per partition
            for kind, k in order:
                if kind == "g":
                    nc.sync.dma_start(wgt0[:, k * 1024:(k + 1) * 1024],
                                      wg_d[0, :, k * 1024:(k + 1) * 1024])
                elif kind == "u":
                    nc.sync.dma_start(wut0[:, k * 1024:(k + 1) * 1024],
                                      wu_d[0, :, k * 1024:(k + 1) * 1024])
                else:
                    nc.sync.dma_start(xt_flat[:, k * XB:(k + 1) * XB],
                                      xt_d[:, k * XB:(k + 1) * XB])
            cmb = xtp.tile([128, NCT], F32, tag="cmb")
            for c in range(NCT):
                w = 128 if c < NCT - 1 else C_LAST
                nc.sync.dma_start(cmb[:w, c:c + 1], cmb_d[c * 128:c * 128 + w, :])

            wd_tiles = []
            ht_tiles = []

            # Phase A: hT[i] = silu(Wg[:,i]^T x^T) * (Wu[:,i]^T x^T), [128, C]
            # Each matmul output must stay inside one 2KB PSUM bank and
            # start=True clears the whole bank, so the C free dim is split
            # into two single-bank tiles (both >=256 keeps fp32r full rate).
            with tc.tile_pool(name="psA", bufs=2, space="PSUM") as psA:
                for i in range(NI):
                    if i == 0:
                        wg_sl = lambda t: wgt0[:, t * 128:(t + 1) * 128]
                        wu_sl = lambda t: wut0[:, t * 128:(t + 1) * 128]
                    else:
                        wgt = wp.tile([128, H], XDT, tag="wg")
                        wut = wp.tile([128, H], XDT, tag="wu")
                        nc.sync.dma_start(wgt[:], wg_d[i])
                        nc.sync.dma_start(wut[:], wu_d[i])
                        wg_sl = lambda t, w=wgt: w[:, t * 128:(t + 1) * 128]
                        wu_sl = lambda t, w=wut: w[:, t * 128:(t + 1) * 128]
                    pg = [psA.tile([128, w], F32, tag=f"pg{k}", name=f"pg{k}_{i}")
                          for k, (_, w) in enumerate(SPLITS)]
                    pu = [psA.tile([128, w], F32, tag=f"pu{k}", name=f"pu{k}_{i}")
                          for k, (_, w) in enumerate(SPLITS)]
                    for t in range(NT):
                        for k, (lo, w) in enumerate(SPLITS):
                            nc.tensor.matmul(pg[k][:], wg_sl(t),
                                             xt[:, t, lo:lo + w],
                                             start=(t == 0), stop=(t == NT - 1))
                    for t in range(NT):
                        for k, (lo, w) in enumerate(SPLITS):
                            nc.tensor.matmul(pu[k][:], wu_sl(t),
                                             xt[:, t, lo:lo + w],
                                             start=(t == 0), stop=(t == NT - 1))
                    tmp = mp.tile([128, C], F32, tag="tmp")
                    ht = htp.tile([128, C], DDT, tag="ht")
                    for k, (lo, w) in enumerate(SPLITS):
                        nc.scalar.activation(tmp[:, lo:lo + w], pg[k][:],
                                             mybir.ActivationFunctionType.Silu)
                        nc.vector.tensor_mul(ht[:, lo:lo + w], tmp[:, lo:lo + w],
                                             pu[k][:])
                    ht_tiles.append(ht)

                    # Trickle-in the down-proj weights during phase A.
                    wdt = wdp.tile([128, H], DDT, tag="wd")
                    nc.sync.dma_start(wdt[:], wd_d[i * 128:(i + 1) * 128, :])
                    wd_tiles.append(wdt)

            # Phase B: y[c-block] = sum_i hT[i][:, c-block]^T @ Wd[i], scaled
            # by the per-token combine weight.
            with tc.tile_pool(name="psB", bufs=2, space="PSUM") as psB:
                for c in range(NCT):
                    cw = 128 if c < NCT - 1 else C_LAST
                    po = psB.tile([cw, H], F32, tag="po", name=f"po_{c}")
                    cs = slice(c * 128, c * 128 + cw)
                    for i in range(NI):
                        for n in range(4):
                            ns = slice(n * 512, (n + 1) * 512)
                            nc.tensor.matmul(po[:, ns], ht_tiles[i][:, cs],
                                             wd_tiles[i][:, ns],
                                             start=(i == 0), stop=(i == NI - 1))
                    ot = op.tile([cw, H], F32, tag="ot", name=f"ot_{c}")
                    for n in range(4):
                        ns = slice(n * 512, (n + 1) * 512)
                        if n % 2 == 0:
                            nc.vector.tensor_scalar_mul(ot[:, ns], po[:, ns],
                                                        cmb[:cw, c:c + 1])
                        else:
                            nc.scalar.activation(
                                ot[:, ns], po[:, ns],
                                mybir.ActivationFunctionType.Copy,
                                scale=cmb[:cw, c:c + 1])
                        nc.sync.dma_start(y_d[cs, ns], ot[:, ns])

    nc.compile()
    return nc


def _route(X: np.ndarray, Wr: np.ndarray):
    """Host router: top-2 of softmax(X @ Wr), renormalized over the top-2."""
    logits = X.astype(np.float64) @ Wr.astype(np.float64)
    order = np.argsort(-logits, axis=1)
    top1, top2 = order[:, 0], order[:, 1]
    rows = np.arange(len(X))
    l1, l2 = logits[rows, top1], logits[rows, top2]
    e21 = np.exp(l2 - l1)
    w1 = 1.0 / (1.0 + e21)
    w2 = e21 / (1.0 + e21)
    return top1, top2, w1.astype(np.float32), w2.astype(np.float32)


def _reference_numpy(hidden_states, w_router, w_gate, w_up, w_down):
    X = np.asarray(hidden_states, np.float32)
    top1, top2, w1, w2 = _route(X, np.asarray(w_router, np.float32))
    out = np.zeros((T, H), np.float32)
    for e in range(E):
        sel = np.where((top1 == e) | (top2 == e))[0]
        if len(sel) == 0:
            continue
        w = np.where(top1[sel] == e, w1[sel], w2[sel])[:, None]
        x = X[sel]
        h = (x @ w_gate[e])
        h = (h / (1.0 + np.exp(-h))) * (x @ w_up[e]) * w
        out[sel] += h @ w_down[e]
    return out


def _make_in_maps(X, Wg, Wu, Wd, sels, wts):
    Xq = _quant_x(X)
    in_maps = []
    for e in range(E):
        sel, w = sels[e], wts[e]
        n = len(sel)
        xt = np.zeros((C, H), Xq.dtype)
        xt[:n] = Xq[sel]
        # [C, H] -> [128, NT*C]: partition p holds x[token c, t*128+p]
        xt = xt.T.reshape(NT, 128, C).transpose(1, 0, 2).reshape(128, NT * C)
        cmb = np.zeros((C, 1), np.float32)
        cmb[:n, 0] = w
        wg_sw = (_quant_x(Wg[e]).reshape(NT, 128, NI, 128)
                 .transpose(2, 1, 0, 3).reshape(NI, 128, H))
        wu_sw = (_quant_x(Wu[e]).reshape(NT, 128, NI, 128)
                 .transpose(2, 1, 0, 3).reshape(NI, 128, H))
        wd_q = _quant_wd(Wd[e])
        in_maps.append({
            "xt": np.ascontiguousarray(xt),
            "wg": np.ascontiguousarray(wg_sw),
            "wu": np.ascontiguousarray(wu_sw),
            "wd": wd_q,
            "cmb": cmb,
        })
    return in_maps


def kernel(hidden_states, w_router, w_gate, w_up, w_down):
    X = np.ascontiguousarray(hidden_states, dtype=np.float32)
    Wr = np.ascontiguousarray(w_router, dtype=np.float32)
    Wg = np.ascontiguousarray(w_gate, dtype=np.float32)
    Wu = np.ascontiguousarray(w_up, dtype=np.float32)
    Wd = np.ascontiguousarray(w_down, dtype=np.float32)

    top1, top2, w1, w2 = _route(X, Wr)
    sels, wts = [], []
    for e in range(E):
        sel = np.where((top1 == e) | (top2 == e))[0]
        sels.append(sel)
        wts.append(np.where(top1[sel] == e, w1[sel], w2[sel]))
    if max(len(s) for s in sels) > C:
        # Capacity overflow (cannot happen for the reference input
        # distribution); fall back to a host implementation.
        return _reference_numpy(X, Wr, Wg, Wu, Wd)

    if "nc" not in _CACHE:
        _CACHE["nc"] = _build()
    nc = _CACHE["nc"]

    in_maps = _make_in_maps(X, Wg, Wu, Wd, sels, wts)
    res = bass_utils.run_bass_kernel_spmd(nc, in_maps, list(range(E)))

    out = np.zeros((T, H), np.float32)
    for e in range(E):
        sel = sels[e]
        out[sel] += res.results[e]["y"][:len(sel)]
    return out


# revision 13
# speedup vs baseline: 1.2413x; 1.0163x over previous
"""DeepSeek-V2 MoE layer (T=2048, H=2048, I=1408, E=8, top-2) on 8 TRN2 cores.

Strategy: expert parallelism. The router (67 MFLOP, 0.06% of total work) runs
on the host to produce the token->expert dispatch; each NeuronCore runs one
expert's gate/up/down GEMMs over the tokens routed to it (padded to a fixed
capacity C), with the top-2 combine weight folded into the output. The host
scatter-adds the per-expert outputs back into the full [T, H] output.

Matmuls run as fp32r (TF32-like, 11 mantissa bits, full PE rate). Inputs are
pre-rounded to the fp32r-representable subset on the host so tiles can be
DMA'd directly (the BIR verifier requires fp32r matmul operands to come from
a rounding producer or be declared fp32r end-to-end).
"""
import sys

_TRN = "/opt/trn_rl_repo"
if _TRN not in sys.path:
    sys.path.insert(0, _TRN)

import numpy as np

import concourse.bacc as bacc
import concourse.mybir as mybir
import concourse.tile as tile
from concourse import bass_utils

T, H, I, E = 2048, 2048, 1408, 8
C = 560                       # per-expert token capacity (actual max count: 545)
NT, NI = H // 128, I // 128   # 16, 11
NCT = (C + 127) // 128        # 5 c-blocks; the last one is partial
C_LAST = C - 128 * (NCT - 1)  # 64
F32 = mybir.dt.float32
SPLITS = ((0, 304), (304, 256))   # phase-A free-dim split: single-bank PSUM tiles

# All matmul operands run in fp16 (10 mantissa bits, full PE rate, FWL
# background weight loads): fp32-class weights would make phase A
# HBM-bound (~390 GB/s needed vs ~358 available) and fp32r stationaries
# serialize their weight loads. fp32 accumulation throughout; measured
# end-to-end rel err ~1e-3 absmax-relative.
XDT = mybir.dt.float16
DDT = mybir.dt.float16

_CACHE = {}


def _round_f32r(x: np.ndarray) -> np.ndarray:
    """Round fp32 to the fp32r-representable subset (RNE to 11 mantissa bits)."""
    u = np.ascontiguousarray(x, dtype=np.float32).view(np.uint32).astype(np.uint64)
    u = u + 0x7FF + ((u >> 12) & 1)
    return (u & np.uint64(0xFFFFF000)).astype(np.uint32).view(np.float32)


def _quant_x(x):
    return np.ascontiguousarray(x, dtype=np.float32).astype(np.float16)


def _quant_wd(x):
    return np.ascontiguousarray(x, dtype=np.float32).astype(np.float16)


def _build():
    nc = bacc.Bacc("TRN2", target_bir_lowering=False, debug=False, num_devices=8)
    xt_d = nc.dram_tensor("xt", [128, NT * C], XDT, kind="ExternalInput").ap()
    wg_d = nc.dram_tensor("wg", [NI, 128, H], XDT, kind="ExternalInput").ap()
    wu_d = nc.dram_tensor("wu", [NI, 128, H], XDT, kind="ExternalInput").ap()
    wd_d = nc.dram_tensor("wd", [I, H], DDT, kind="ExternalInput").ap()
    cmb_d = nc.dram_tensor("cmb", [C, 1], F32, kind="ExternalInput").ap()
    y_d = nc.dram_tensor("y", [C, H], F32, kind="ExternalOutput").ap()

    with tile.TileContext(nc) as tc:
        with (
            tc.tile_pool(name="xtp", bufs=1) as xtp,
            tc.tile_pool(name="wp", bufs=3) as wp,
            tc.tile_pool(name="htp", bufs=NI) as htp,
            tc.tile_pool(name="wdp", bufs=NI) as wdp,
            tc.tile_pool(name="mp", bufs=2) as mp,
            tc.tile_pool(name="op", bufs=2) as op,
        ):
            # Front loads, interleaved so the PE can start as early as
            # possible: the first phase-A iteration's weights arrive in
            # 512-column chunks alongside the first x^T tiles.
            xt = xtp.tile([128, NT, C], XDT, tag="xt")
            xt_flat = xt.rearrange("p t c -> p (t c)")
            wgt0 = wp.tile([128, H], XDT, tag="wg", name="wgt0")
            wut0 = wp.tile([128, H], XDT, tag="wu", name="wut0")
            # x^T arrives in 8 two-h-block chunks; the first iteration's
            # weights arrive in 1024-column chunks, interleaved.
            order = [("g", 0), ("x", 0), ("g", 1), ("x", 1), ("g", 2), ("g", 3),
                     ("u", 0), ("x", 2), ("u", 1), ("x", 3)] + \
                    [("x", b) for b in range(4, 8)]
            GCH = {0: (0, 512), 1: (512, 512), 2: (1024, 512), 3: (1536, 512)}
            XB = 2 * C  # elements per xt chunk per partition
            for kind, k in order:
                if kind == "g":
                    lo, w = GCH[k]
                    nc.sync.dma_start(wgt0[:, lo:lo + w], wg_d[0, :, lo:lo + w])
                elif kind == "u":
                    nc.sync.dma_start(wut0[:, k * 1024:(k + 1) * 1024],
                                      wu_d[0, :, k * 1024:(k + 1) * 1024])
                else:
                    nc.sync.dma_start(xt_flat[:, k * XB:(k + 1) * XB],
                                      xt_d[:, k * XB:(k + 1) * XB])
            cmb = xtp.tile([128, NCT], F32, tag="cmb")
            for c in range(NCT):
                w = 128 if c < NCT - 1 else C_LAST
                nc.sync.dma_start(cmb[:w, c:c + 1], cmb_d[c * 128:c * 128 + w, :])

            wd_tiles = []
            ht_tiles = []

            # Phase A: hT[i] = silu(Wg[:,i]^T x^T) * (Wu[:,i]^T x^T), [128, C]
            # Each matmul output must stay inside one 2KB PSUM bank and
            # start=True clears the whole bank, so the C free dim is split
            # into two single-bank tiles (both >=256 keeps fp32r full rate).
            with tc.tile_pool(name="psA", bufs=2, space="PSUM") as psA:
                for i in range(NI):
                    if i == 0:
                        wg_sl = lambda t: wgt0[:, t * 128:(t + 1) * 128]
                        wu_sl = lambda t: wut0[:, t * 128:(t + 1) * 128]
                    else:
                        wgt = wp.tile([128, H], XDT, tag="wg")
                        wut = wp.tile([128, H], XDT, tag="wu")
                        nc.sync.dma_start(wgt[:], wg_d[i])
                        nc.sync.dma_start(wut[:], wu_d[i])
                        wg_sl = lambda t, w=wgt: w[:, t * 128:(t + 1) * 128]
                        wu_sl = lambda t, w=wut: w[:, t * 128:(t + 1) * 128]
                    pg = [psA.tile([128, w], F32, tag=f"pg{k}", name=f"pg{k}_{i}")
                          for k, (_, w) in enumerate(SPLITS)]
                    pu = [psA.tile([128, w], F32, tag=f"pu{k}", name=f"pu{k}_{i}")
                          for k, (_, w) in enumerate(SPLITS)]
                    for t in range(NT):
                        for k, (lo, w) in enumerate(SPLITS):
                            nc.tensor.matmul(pg[k][:], wg_sl(t),
                                             xt[:, t, lo:lo + w],
                                             start=(t == 0), stop=(t == NT - 1))
                    for t in range(NT):
                        for k, (lo, w) in enumerate(SPLITS):
                            nc.tensor.matmul(pu[k][:], wu_sl(t),
                                             xt[:, t, lo:lo + w],
                                             start=(t == 0), stop=(t == NT - 1))
                    tmp = mp.tile([128, C], F32, tag="tmp")
                    ht = htp.tile([128, C], DDT, tag="ht")
                    for k, (lo, w) in enumerate(SPLITS):
                        nc.scalar.activation(tmp[:, lo:lo + w], pg[k][:],
                                             mybir.ActivationFunctionType.Silu)
                        nc.vector.tensor_mul(ht[:, lo:lo + w], tmp[:, lo:lo + w],
                                             pu[k][:])
                    ht_tiles.append(ht)

                    # Trickle-in the down-proj weights during phase A.
                    wdt = wdp.tile([128, H], DDT, tag="wd")
                    nc.sync.dma_start(wdt[:], wd_d[i * 128:(i + 1) * 128, :])
                    wd_tiles.append(wdt)

            # Phase B: y[c-block] = sum_i hT[i][:, c-block]^T @ Wd[i], scaled
            # by the per-token combine weight.
            with tc.tile_pool(name="psB", bufs=2, space="PSUM") as psB:
                for c in range(NCT):
                    cw = 128 if c < NCT - 1 else C_LAST
                    po = psB.tile([cw, H], F32, tag="po", name=f"po_{c}")
                    cs = slice(c * 128, c * 128 + cw)
                    for i in range(NI):
                        for n in range(4):
                            ns = slice(n * 512, (n + 1) * 512)
                            nc.tensor.matmul(po[:, ns], ht_tiles[i][:, cs],
                                             wd_tiles[i][:, ns],
                                             start=(i == 0), stop=(i == NI - 1))
                    ot = op.tile([cw, H], F32, tag="ot", name=f"ot_{c}")
                    for n in range(4):
                        ns = slice(n * 512, (n + 1) * 512)
                        if n % 2 == 0:
                            nc.vector.tensor_scalar_mul(ot[:, ns], po[:, ns],
                                                        cmb[:cw, c:c + 1])
                        else:
                            nc.scalar.activation(
                                ot[:, ns], po[:, ns],
                                mybir.ActivationFunctionType.Copy,
                                scale=cmb[:cw, c:c + 1])
                        nc.sync.dma_start(y_d[cs, ns], ot[:, ns])

    nc.compile()
    return nc


def _route(X: np.ndarray, Wr: np.ndarray):
    """Host router: top-2 of softmax(X @ Wr), renormalized over the top-2."""
    logits = X.astype(np.float64) @ Wr.astype(np.float64)
    order = np.argsort(-logits, axis=1)
    top1, top2 = order[:, 0], order[:, 1]
    rows = np.arange(len(X))
    l1, l2 = logits[rows, top1], logits[rows, top2]
    e21 = np.exp(l2 - l1)
    w1 = 1.0 / (1.0 + e21)
    w2 = e21 / (1.0 + e21)
    return top1, top2, w1.astype(np.float32), w2.astype(np.float32)


def _reference_numpy(hidden_states, w_router, w_gate, w_up, w_down):
    X = np.asarray(hidden_states, np.float32)
    top1, top2, w1, w2 = _route(X, np.asarray(w_router, np.float32))
    out = np.zeros((T, H), np.float32)
    for e in range(E):
        sel = np.where((top1 == e) | (top2 == e))[0]
        if len(sel) == 0:
            continue
        w = np.where(top1[sel] == e, w1[sel], w2[sel])[:, None]
        x = X[sel]
        h = (x @ w_gate[e])
        h = (h / (1.0 + np.exp(-h))) * (x @ w_up[e]) * w
        out[sel] += h @ w_down[e]
    return out


def _make_in_maps(X, Wg, Wu, Wd, sels, wts):
    Xq = _quant_x(X)
    in_maps = []
    for e in range(E):
        sel, w = sels[e], wts[e]
        n = len(sel)
        xt = np.zeros((C, H), Xq.dtype)
        xt[:n] = Xq[sel]
        # [C, H] -> [128, NT*C]: partition p holds x[token c, t*128+p]
        xt = xt.T.reshape(NT, 128, C).transpose(1, 0, 2).reshape(128, NT * C)
        cmb = np.zeros((C, 1), np.float32)
        cmb[:n, 0] = w
        wg_sw = (_quant_x(Wg[e]).reshape(NT, 128, NI, 128)
                 .transpose(2, 1, 0, 3).reshape(NI, 128, H))
        wu_sw = (_quant_x(Wu[e]).reshape(NT, 128, NI, 128)
                 .transpose(2, 1, 0, 3).reshape(NI, 128, H))
        wd_q = _quant_wd(Wd[e])
        in_maps.append({
            "xt": np.ascontiguousarray(xt),
            "wg": np.ascontiguousarray(wg_sw),
            "wu": np.ascontiguousarray(wu_sw),
            "wd": wd_q,
            "cmb": cmb,
        })
    return in_maps


def kernel(hidden_states, w_router, w_gate, w_up, w_down):
    X = np.ascontiguousarray(hidden_states, dtype=np.float32)
    Wr = np.ascontiguousarray(w_router, dtype=np.float32)
    Wg = np.ascontiguousarray(w_gate, dtype=np.float32)
    Wu = np.ascontiguousarray(w_up, dtype=np.float32)
    Wd = np.ascontiguousarray(w_down, dtype=np.float32)

    top1, top2, w1, w2 = _route(X, Wr)
    sels, wts = [], []
    for e in range(E):
        sel = np.where((top1 == e) | (top2 == e))[0]
        sels.append(sel)
        wts.append(np.where(top1[sel] == e, w1[sel], w2[sel]))
    if max(len(s) for s in sels) > C:
        # Capacity overflow (cannot happen for the reference input
        # distribution); fall back to a host implementation.
        return _reference_numpy(X, Wr, Wg, Wu, Wd)

    if "nc" not in _CACHE:
        _CACHE["nc"] = _build()
    nc = _CACHE["nc"]

    in_maps = _make_in_maps(X, Wg, Wu, Wd, sels, wts)
    res = bass_utils.run_bass_kernel_spmd(nc, in_maps, list(range(E)))

    out = np.zeros((T, H), np.float32)
    for e in range(E):
        sel = sels[e]
        out[sel] += res.results[e]["y"][:len(sel)]
    return out
